# revision 19
# baseline (speedup 1.0000x reference)
"""Trainium2 Bass kernel for additive (Bahdanau) attention.

Math (per batch b):
  kp[k,d] = c_key[k,:,b] @ W1[:H]   (+ b1)
  qp[q,d] = query[q,:,b] @ W1[H:]
  score[k,q] = sum_d W2[d] * tanh(kp[k,d] + qp[q,d])   (+ b2, softmax-invariant)
  score[masked k] = -inf ;  prob = softmax_k(score)
  ant[q,h] = sum_k prob[k,q] * c_key[k,b,h]

Sharding: data-parallel over batch B=8 across the 8 cores (one batch each).

Per-core dataflow (all engines pipelined by the Tile framework):
  - transpose c_key/query blocks on PE (via identity matmul) to get h on
    partitions, then compute kp^T[d,k] / qp^T[d,q] with W1 as the stationary
    operand (d on partitions, 4 chunks of 128).
  - big tanh tiles [128(d-chunk), 32(q) x 256(k)]: DVE tensor_scalar_add
    broadcasts qp^T[d,q] (per-partition scalar) over k; ACT does tanh on
    the whole [128, 8192] tile (ACT is the bottleneck: 16.8M tanh/core).
  - W2-weighted reduction over d via m=1 matmuls col-tiled to PSUM
    partitions {0,32,64,96}; the mask bias is pre-loaded into PSUM with a
    K=1 ones-matmul (start=True) so masking costs nothing extra.
  - scores are drained PSUM->SBUF, scattered to [q=128, k=256] layout via a
    small DRAM bounce, then softmax (no max-subtraction needed:
    |score| <= sum|W2| ~ 18; masked entries are -1e30 -> exp == 0 exactly).
  - prob is PE-transposed to [k,q] and used as the stationary operand
    against c_key in natural [k,h] layout for the final ant matmul.
"""

import os
import sys

if "/opt/trn_rl_repo" not in sys.path:
    sys.path.insert(0, "/opt/trn_rl_repo")

import ml_dtypes
import numpy as np

import concourse.bacc as bacc
import concourse.bass as bass
import concourse.mybir as mybir
import concourse.tile as tile
from concourse.bass_utils import run_bass_kernel_spmd

T_K, T_Q, B, H = 256, 128, 8, 512
NC = 8          # cores
DC = 4          # d-chunks of 128 (H / 128)
KB = 2          # k-blocks of 128 (T_K / 128)
QB = 4          # q-blocks of 32  (T_Q / 32)
QPB = 32        # q per block
NROW = 16       # score rows (q-pairs) per q-block
F32 = mybir.dt.float32
BF16 = mybir.dt.bfloat16
AF = mybir.ActivationFunctionType
NEG = -1.0e30
# dtype experiments: KP_BF16 stores the k-projection in bf16 (halves the
# DVE read cost of the broadcast adds); TT_BF16 also stores the tanh tiles
# + W2 in bf16 (4x DVE add mode, halves PE matmul passes).
KP_BF16 = os.environ.get("KP_BF16", "0") == "1"
TT_BF16 = os.environ.get("TT_BF16", "0") == "1"
# how many of the 32 q-slices per tile run as fused tanh(kp+bias) on ACT
# (balances the DVE broadcast-add load against ACT slack)
FUSE_Q = int(os.environ.get("FUSE_Q", "6"))
KP_DT = BF16 if KP_BF16 else F32
TT_DT = BF16 if TT_BF16 else F32


def build_program(trace_sim: bool = False) -> bacc.Bacc:
    nc = bacc.Bacc(
        "TRN2", target_bir_lowering=False, debug=False, num_devices=NC
    )

    # ---- DRAM I/O (per-core views; host passes per-batch slices) ----
    ck_d = nc.dram_tensor("ck", [T_K, H], F32, kind="ExternalInput")
    q_d = nc.dram_tensor("qin", [T_Q, H], F32, kind="ExternalInput")
    w1a_d = nc.dram_tensor("w1a", [H, H], F32, kind="ExternalInput")
    w1b_d = nc.dram_tensor("w1b", [H, H], F32, kind="ExternalInput")
    w2t_d = nc.dram_tensor("w2t", [128, DC], TT_DT, kind="ExternalInput")
    b1t_d = nc.dram_tensor("b1t", [128, DC], F32, kind="ExternalInput")
    madd_d = nc.dram_tensor("madd", [1, 256], F32, kind="ExternalInput")
    ident_d = nc.dram_tensor("ident", [128, 128], F32, kind="ExternalInput")
    ones_d = nc.dram_tensor("ones", [1, 128], F32, kind="ExternalInput")

    ant_d = nc.dram_tensor("ant", [T_Q, H], F32, kind="ExternalOutput")
    prob_d = nc.dram_tensor("prob_t", [T_K, T_Q], F32, kind="ExternalOutput")

    # scratch for the score partition-scatter (qb, r_hi, r_lo, j, k)
    scr_d = nc.dram_tensor("scr", [QB, 4, 4, 2, 256], F32)

    with tile.TileContext(nc, trace_sim=trace_sim) as tc:
        with (
            tc.tile_pool(name="const", bufs=1) as constp,
            tc.tile_pool(name="work", bufs=1) as workp,
        ):
            # ---- load inputs (transpose sources first so PE starts early) ----
            ident = constp.tile([128, 128], F32)
            nc.sync.dma_start(ident[:], ident_d[:])
            ck = [workp.tile([128, H], F32, tag=f"ck{i}", name=f"ck{i}") for i in range(KB)]
            for kb in range(KB):
                nc.sync.dma_start(ck[kb][:], ck_d[kb * 128 : (kb + 1) * 128, :])
            qsb = workp.tile([128, H], F32, tag="qsb")
            nc.sync.dma_start(qsb[:], q_d[:])

            w1a = constp.tile([128, 4 * H], F32, tag="w1a")
            w1b = constp.tile([128, 4 * H], F32, tag="w1b")
            for hc in range(4):
                nc.sync.dma_start(
                    w1a[:, hc * H : (hc + 1) * H],
                    w1a_d[hc * 128 : (hc + 1) * 128, :],
                )
                nc.sync.dma_start(
                    w1b[:, hc * H : (hc + 1) * H],
                    w1b_d[hc * 128 : (hc + 1) * 128, :],
                )

            ones = constp.tile([1, 128], F32)
            nc.sync.dma_start(ones[:], ones_d[:])
            madd = constp.tile([1, 256], F32)
            nc.sync.dma_start(madd[:], madd_d[:])
            w2t = constp.tile([128, DC], TT_DT)
            nc.sync.dma_start(w2t[:], w2t_d[:])
            b1t = constp.tile([128, DC], F32)
            nc.sync.dma_start(b1t[:], b1t_d[:])

            # ---- prep: transposes and projections ----
            ckT = [workp.tile([128, T_K], F32, tag=f"ckT{i}", name=f"ckT{i}") for i in range(DC)]
            qT = [workp.tile([128, 128], F32, tag=f"qT{i}", name=f"qT{i}") for i in range(DC)]
            kp = [workp.tile([128, T_K], KP_DT, tag=f"kp{i}", name=f"kp{i}") for i in range(DC)]
            qp = [workp.tile([128, T_Q], F32, tag=f"qp{i}", name=f"qp{i}") for i in range(DC)]

            mask_sb = workp.tile([128, 256], F32, tag="mask_sb")

            with tc.tile_pool(
                name="psum_prep", bufs=1, space=bass.MemorySpace.PSUM
            ) as pp:
                if True:
                    # mask bias broadcast to all 128 q-partitions (built once)
                    pm = pp.tile([128, 256], F32, tag="tp", bufs=2)
                    nc.tensor.matmul(pm[:], ones[:], madd[:], start=True, stop=True)
                    nc.vector.tensor_copy(mask_sb[:], pm[:])
                    for hc in range(4):
                        for kb in range(KB):
                            tp = pp.tile([128, 256], F32, tag="tp", bufs=2)
                            nc.tensor.transpose(
                                tp[:, :128],
                                ck[kb][:, hc * 128 : (hc + 1) * 128],
                                ident[:],
                            )
                            nc.vector.tensor_copy(
                                ckT[hc][:, kb * 128 : (kb + 1) * 128], tp[:, :128]
                            )
                        tp = pp.tile([128, 256], F32, tag="tp", bufs=2)
                        nc.tensor.transpose(
                            tp[:, :128], qsb[:, hc * 128 : (hc + 1) * 128], ident[:]
                        )
                        nc.vector.tensor_copy(qT[hc][:], tp[:, :128])

                    for dc in range(DC):
                        pk = pp.tile([128, T_K], F32, tag="pk")
                        for hc in range(4):
                            nc.tensor.matmul(
                                pk[:],
                                w1a[:, hc * H + dc * 128 : hc * H + (dc + 1) * 128],
                                ckT[hc][:],
                                start=(hc == 0),
                                stop=(hc == 3),
                            )
                        # fold b1 while draining PSUM -> SBUF
                        nc.vector.tensor_scalar_add(
                            kp[dc][:], pk[:], b1t[:, dc : dc + 1]
                        )

                        pq = pp.tile([128, T_Q], F32, tag="pq")
                        for hc in range(4):
                            nc.tensor.matmul(
                                pq[:],
                                w1b[:, hc * H + dc * 128 : hc * H + (dc + 1) * 128],
                                qT[hc][:],
                                start=(hc == 0),
                                stop=(hc == 3),
                            )
                        nc.vector.tensor_copy(qp[dc][:], pq[:])

            # ---- main loop: tanh tiles + W2 reduction into PSUM ----
            # double-buffered score PSUM (2 x 4 banks), memset once so reads
            # of never-written partitions are defined
            S = workp.tile([128, T_K], F32, tag="S")  # scores [q, k]

            with (
                tc.tile_pool(
                    name="psum_s", bufs=1, space=bass.MemorySpace.PSUM
                ) as psp,
                tc.tile_pool(name="tt", bufs=3) as ttp,
                tc.tile_pool(name="drain", bufs=2) as drp,
            ):
                    spss = [
                        psp.tile([128, 4 * 512], F32, tag=f"sps{i}", name=f"sps{i}")
                        for i in range(2)
                    ]
                    nc.vector.memset(spss[0][:], 0.0)
                    nc.vector.memset(spss[1][:], 0.0)
                    for qb in range(QB):
                        sps = spss[qb % 2]
                        for c in range(DC):
                            tt = ttp.tile([128, QPB * 256], TT_DT, tag="tt")
                            for ql in range(FUSE_Q):
                                q = qb * QPB + ql
                                nc.scalar.activation(
                                    tt[:, ql * 256 : (ql + 1) * 256],
                                    kp[c][:],
                                    AF.Tanh,
                                    bias=qp[c][:, q : q + 1],
                                )
                            for ql in range(FUSE_Q, QPB):
                                q = qb * QPB + ql
                                nc.vector.tensor_scalar_add(
                                    tt[:, ql * 256 : (ql + 1) * 256],
                                    kp[c][:],
                                    qp[c][:, q : q + 1],
                                )
                            nc.scalar.activation(
                                tt[:, FUSE_Q * 256 :], tt[:, FUSE_Q * 256 :], AF.Tanh
                            )
                            for r in range(NROW):
                                grp = r & 3
                                bank = r >> 2
                                nc.tensor.matmul(
                                    sps[
                                        32 * grp : 32 * grp + 1,
                                        bank * 512 : (bank + 1) * 512,
                                    ],
                                    w2t[:, c : c + 1],
                                    tt[:, r * 512 : (r + 1) * 512],
                                    start=(c == 0),
                                    stop=(c == DC - 1),
                                    tile_position=(0, 32 * grp),
                                    skip_group_check=True,
                                )
                        # drain full psum block, bounce through DRAM to scatter
                        sdr = drp.tile([128, 4 * 512], F32, tag="sdr")
                        nc.vector.tensor_copy(sdr[:], sps[:])
                        for rl in range(4):
                            nc.sync.dma_start(
                                scr_d[qb][:, rl, :, :],
                                sdr[32 * rl : 32 * rl + 1, :].rearrange(
                                    "p (rh j k) -> p rh j k", rh=4, j=2
                                ),
                            )
                        nc.sync.dma_start(
                            S[qb * QPB : (qb + 1) * QPB, :],
                            scr_d[qb].rearrange("rh rl j k -> (rh rl j) k"),
                        )

            # ---- softmax over k (free dim); no max-subtraction needed ----
            with tc.tile_pool(name="smax", bufs=1) as smp:
                nc.vector.tensor_tensor(
                    S[:], S[:], mask_sb[:], mybir.AluOpType.add
                )
                nc.scalar.activation(S[:], S[:], AF.Exp)
                z = smp.tile([128, 1], F32)
                nc.vector.tensor_reduce(
                    z[:], S[:], mybir.AxisListType.X, mybir.AluOpType.add
                )
                r = smp.tile([128, 1], F32)
                nc.vector.reciprocal(r[:], z[:])
                nc.vector.tensor_scalar_mul(S[:], S[:], r[:, 0:1])

            # ---- prob transpose + ant matmul + outputs ----
            with (
                tc.tile_pool(
                    name="psum_t", bufs=2, space=bass.MemorySpace.PSUM
                ) as ptp,
                tc.tile_pool(name="tail", bufs=1) as tlp,
            ):
                pT = [tlp.tile([128, 128], F32, tag=f"pT{i}", name=f"pT{i}") for i in range(KB)]
                for kb in range(KB):
                    tp = ptp.tile([128, 128], F32, tag="tp2")
                    nc.tensor.transpose(
                        tp[:], S[:, kb * 128 : (kb + 1) * 128], ident[:]
                    )
                    nc.vector.tensor_copy(pT[kb][:], tp[:])
                    nc.sync.dma_start(prob_d[kb * 128 : (kb + 1) * 128, :], pT[kb][:])

                pant = ptp.tile([128, H], F32, tag="pant")
                for kb in range(KB):
                    nc.tensor.matmul(
                        pant[:],
                        pT[kb][:],
                        ck[kb][:],
                        start=(kb == 0),
                        stop=(kb == KB - 1),
                    )
                asb = tlp.tile([128, H], F32, tag="asb")
                nc.vector.tensor_copy(asb[:], pant[:])
                nc.sync.dma_start(ant_d[:], asb[:])

    nc.compile()
    return nc


_NC_CACHE = None


def _get_program():
    global _NC_CACHE
    if _NC_CACHE is None:
        _NC_CACHE = build_program()
    return _NC_CACHE


def make_in_maps(query, c_key, mask, W1, b1, W2, b2):
    query = np.asarray(query, dtype=np.float32)
    c_key = np.asarray(c_key, dtype=np.float32)
    mask = np.asarray(mask)
    W1 = np.asarray(W1, dtype=np.float32)
    b1 = np.asarray(b1, dtype=np.float32)
    W2 = np.asarray(W2, dtype=np.float32)

    w1a = np.ascontiguousarray(W1[:H])
    w1b = np.ascontiguousarray(W1[H:])
    w2t = np.ascontiguousarray(W2[:, 0].reshape(DC, 128).T)
    if TT_BF16:
        w2t = w2t.astype(ml_dtypes.bfloat16)
    b1t = np.ascontiguousarray(b1.reshape(DC, 128).T)
    ident = np.eye(128, dtype=np.float32)
    ones = np.ones((1, 128), dtype=np.float32)

    in_maps = []
    for b in range(NC):
        madd = np.where(mask[:, b], np.float32(NEG), np.float32(0.0)).astype(
            np.float32
        )
        in_maps.append(
            {
                "ck": np.ascontiguousarray(c_key[:, b, :]),
                "qin": np.ascontiguousarray(query[:, b, :]),
                "w1a": w1a,
                "w1b": w1b,
                "w2t": w2t,
                "b1t": b1t,
                "madd": madd[None, :],
                "ident": ident,
                "ones": ones,
            }
        )
    return in_maps


def assemble(results):
    ant = np.stack([results[b]["ant"] for b in range(NC)], axis=1)
    prob = np.stack([results[b]["prob_t"] for b in range(NC)], axis=-1)
    return ant, prob


def kernel(query, c_key, mask, W1, b1, W2, b2):
    nc = _get_program()
    in_maps = make_in_maps(query, c_key, mask, W1, b1, W2, b2)
    res = run_bass_kernel_spmd(nc, in_maps, list(range(NC))).results
    return assemble(res)


# revision 20
# speedup vs baseline: 1.0284x; 1.0284x over previous
"""Trainium2 Bass kernel for additive (Bahdanau) attention.

Math (per batch b):
  kp[k,d] = c_key[k,:,b] @ W1[:H]   (+ b1)
  qp[q,d] = query[q,:,b] @ W1[H:]
  score[k,q] = sum_d W2[d] * tanh(kp[k,d] + qp[q,d])   (+ b2, softmax-invariant)
  score[masked k] = -inf ;  prob = softmax_k(score)
  ant[q,h] = sum_k prob[k,q] * c_key[k,b,h]

Sharding: data-parallel over batch B=8 across the 8 cores (one batch each).

Per-core dataflow (all engines pipelined by the Tile framework):
  - transpose c_key/query blocks on PE (via identity matmul) to get h on
    partitions, then compute kp^T[d,k] / qp^T[d,q] with W1 as the stationary
    operand (d on partitions, 4 chunks of 128).
  - big tanh tiles [128(d-chunk), 32(q) x 256(k)]: DVE tensor_scalar_add
    broadcasts qp^T[d,q] (per-partition scalar) over k; ACT does tanh on
    the whole [128, 8192] tile (ACT is the bottleneck: 16.8M tanh/core).
  - W2-weighted reduction over d via m=1 matmuls col-tiled to PSUM
    partitions {0,32,64,96}; the mask bias is pre-loaded into PSUM with a
    K=1 ones-matmul (start=True) so masking costs nothing extra.
  - scores are drained PSUM->SBUF, scattered to [q=128, k=256] layout via a
    small DRAM bounce, then softmax (no max-subtraction needed:
    |score| <= sum|W2| ~ 18; masked entries are -1e30 -> exp == 0 exactly).
  - prob is PE-transposed to [k,q] and used as the stationary operand
    against c_key in natural [k,h] layout for the final ant matmul.
"""

import os
import sys

if "/opt/trn_rl_repo" not in sys.path:
    sys.path.insert(0, "/opt/trn_rl_repo")

import ml_dtypes
import numpy as np

import concourse.bacc as bacc
import concourse.bass as bass
import concourse.mybir as mybir
import concourse.tile as tile
from concourse.bass_utils import run_bass_kernel_spmd

T_K, T_Q, B, H = 256, 128, 8, 512
NC = 8          # cores
DC = 4          # d-chunks of 128 (H / 128)
KB = 2          # k-blocks of 128 (T_K / 128)
QB = 4          # q-blocks of 32  (T_Q / 32)
QPB = 32        # q per block
NROW = 16       # score rows (q-pairs) per q-block
F32 = mybir.dt.float32
BF16 = mybir.dt.bfloat16
AF = mybir.ActivationFunctionType
NEG = -1.0e30
# dtype experiments: KP_BF16 stores the k-projection in bf16 (halves the
# DVE read cost of the broadcast adds); TT_BF16 also stores the tanh tiles
# + W2 in bf16 (4x DVE add mode, halves PE matmul passes).
KP_BF16 = os.environ.get("KP_BF16", "0") == "1"
TT_BF16 = os.environ.get("TT_BF16", "0") == "1"
# how many of the 32 q-slices per tile run as fused tanh(kp+bias) on ACT
# (balances the DVE broadcast-add load against ACT slack)
FUSE_Q = int(os.environ.get("FUSE_Q", "6"))
KP_DT = BF16 if KP_BF16 else F32
TT_DT = BF16 if TT_BF16 else F32


def build_program(trace_sim: bool = False) -> bacc.Bacc:
    nc = bacc.Bacc(
        "TRN2", target_bir_lowering=False, debug=False, num_devices=NC
    )

    # ---- DRAM I/O (per-core views; host passes per-batch slices) ----
    ck_d = nc.dram_tensor("ck", [T_K, H], F32, kind="ExternalInput")
    q_d = nc.dram_tensor("qin", [T_Q, H], F32, kind="ExternalInput")
    w1a_d = nc.dram_tensor("w1a", [H, H], F32, kind="ExternalInput")
    w1b_d = nc.dram_tensor("w1b", [H, H], F32, kind="ExternalInput")
    w2t_d = nc.dram_tensor("w2t", [128, DC], TT_DT, kind="ExternalInput")
    b1t_d = nc.dram_tensor("b1t", [128, DC], F32, kind="ExternalInput")
    madd_d = nc.dram_tensor("madd", [1, 256], F32, kind="ExternalInput")
    ident_d = nc.dram_tensor("ident", [128, 128], F32, kind="ExternalInput")
    ones_d = nc.dram_tensor("ones", [1, 128], F32, kind="ExternalInput")

    ant_d = nc.dram_tensor("ant", [T_Q, H], F32, kind="ExternalOutput")
    prob_d = nc.dram_tensor("prob_t", [T_K, T_Q], F32, kind="ExternalOutput")

    # scratch for the score partition-scatter (qb, r_hi, r_lo, j, k)
    scr_d = nc.dram_tensor("scr", [QB, 4, 4, 2, 256], F32)

    with tile.TileContext(nc, trace_sim=trace_sim) as tc:
        with (
            tc.tile_pool(name="const", bufs=1) as constp,
            tc.tile_pool(name="work", bufs=1) as workp,
        ):
            # ---- load inputs (small constants + transpose sources first
            # so PE's queue head is never blocked on a big W1 transfer) ----
            ident = constp.tile([128, 128], F32)
            nc.sync.dma_start(ident[:], ident_d[:])
            ones = constp.tile([1, 128], F32)
            nc.sync.dma_start(ones[:], ones_d[:])
            madd = constp.tile([1, 256], F32)
            nc.sync.dma_start(madd[:], madd_d[:])
            w2t = constp.tile([128, DC], TT_DT)
            nc.sync.dma_start(w2t[:], w2t_d[:])
            b1t = constp.tile([128, DC], F32)
            nc.sync.dma_start(b1t[:], b1t_d[:])
            ck = [workp.tile([128, H], F32, tag=f"ck{i}", name=f"ck{i}") for i in range(KB)]
            for kb in range(KB):
                nc.sync.dma_start(ck[kb][:], ck_d[kb * 128 : (kb + 1) * 128, :])
            qsb = workp.tile([128, H], F32, tag="qsb")
            nc.sync.dma_start(qsb[:], q_d[:])

            w1a = constp.tile([128, 4 * H], F32, tag="w1a")
            w1b = constp.tile([128, 4 * H], F32, tag="w1b")
            for hc in range(4):
                nc.sync.dma_start(
                    w1a[:, hc * H : (hc + 1) * H],
                    w1a_d[hc * 128 : (hc + 1) * 128, :],
                )
                nc.sync.dma_start(
                    w1b[:, hc * H : (hc + 1) * H],
                    w1b_d[hc * 128 : (hc + 1) * 128, :],
                )

            # ---- prep: transposes and projections ----
            ckT = [workp.tile([128, T_K], F32, tag=f"ckT{i}", name=f"ckT{i}") for i in range(DC)]
            qT = [workp.tile([128, 128], F32, tag=f"qT{i}", name=f"qT{i}") for i in range(DC)]
            kp = [workp.tile([128, T_K], KP_DT, tag=f"kp{i}", name=f"kp{i}") for i in range(DC)]
            qp = [workp.tile([128, T_Q], F32, tag=f"qp{i}", name=f"qp{i}") for i in range(DC)]

            mask_sb = workp.tile([128, 256], F32, tag="mask_sb")

            with tc.tile_pool(
                name="psum_prep", bufs=1, space=bass.MemorySpace.PSUM
            ) as pp:
                if True:
                    # mask bias broadcast to all 128 q-partitions (built once)
                    pm = pp.tile([128, 256], F32, tag="tp", bufs=2)
                    nc.tensor.matmul(pm[:], ones[:], madd[:], start=True, stop=True)
                    nc.vector.tensor_copy(mask_sb[:], pm[:])
                    for hc in range(4):
                        for kb in range(KB):
                            tp = pp.tile([128, 256], F32, tag="tp", bufs=2)
                            nc.tensor.transpose(
                                tp[:, :128],
                                ck[kb][:, hc * 128 : (hc + 1) * 128],
                                ident[:],
                            )
                            nc.vector.tensor_copy(
                                ckT[hc][:, kb * 128 : (kb + 1) * 128], tp[:, :128]
                            )
                        tp = pp.tile([128, 256], F32, tag="tp", bufs=2)
                        nc.tensor.transpose(
                            tp[:, :128], qsb[:, hc * 128 : (hc + 1) * 128], ident[:]
                        )
                        nc.vector.tensor_copy(qT[hc][:], tp[:, :128])

                    for dc in range(DC):
                        pk = pp.tile([128, T_K], F32, tag="pk")
                        for hc in range(4):
                            nc.tensor.matmul(
                                pk[:],
                                w1a[:, hc * H + dc * 128 : hc * H + (dc + 1) * 128],
                                ckT[hc][:],
                                start=(hc == 0),
                                stop=(hc == 3),
                            )
                        # fold b1 while draining PSUM -> SBUF
                        nc.vector.tensor_scalar_add(
                            kp[dc][:], pk[:], b1t[:, dc : dc + 1]
                        )

                        pq = pp.tile([128, T_Q], F32, tag="pq")
                        for hc in range(4):
                            nc.tensor.matmul(
                                pq[:],
                                w1b[:, hc * H + dc * 128 : hc * H + (dc + 1) * 128],
                                qT[hc][:],
                                start=(hc == 0),
                                stop=(hc == 3),
                            )
                        nc.vector.tensor_copy(qp[dc][:], pq[:])

            # ---- main loop: tanh tiles + W2 reduction into PSUM ----
            # double-buffered score PSUM (2 x 4 banks), memset once so reads
            # of never-written partitions are defined
            S = workp.tile([128, T_K], F32, tag="S")  # scores [q, k]

            with (
                tc.tile_pool(
                    name="psum_s", bufs=1, space=bass.MemorySpace.PSUM
                ) as psp,
                tc.tile_pool(name="tt", bufs=3) as ttp,
                tc.tile_pool(name="drain", bufs=2) as drp,
            ):
                    spss = [
                        psp.tile([128, 4 * 512], F32, tag=f"sps{i}", name=f"sps{i}")
                        for i in range(2)
                    ]
                    nc.vector.memset(spss[0][:], 0.0)
                    nc.vector.memset(spss[1][:], 0.0)
                    for qb in range(QB):
                        sps = spss[qb % 2]
                        for c in range(DC):
                            tt = ttp.tile([128, QPB * 256], TT_DT, tag="tt")
                            for ql in range(FUSE_Q):
                                q = qb * QPB + ql
                                nc.scalar.activation(
                                    tt[:, ql * 256 : (ql + 1) * 256],
                                    kp[c][:],
                                    AF.Tanh,
                                    bias=qp[c][:, q : q + 1],
                                )
                            for ql in range(FUSE_Q, QPB):
                                q = qb * QPB + ql
                                nc.vector.tensor_scalar_add(
                                    tt[:, ql * 256 : (ql + 1) * 256],
                                    kp[c][:],
                                    qp[c][:, q : q + 1],
                                )
                            nc.scalar.activation(
                                tt[:, FUSE_Q * 256 :], tt[:, FUSE_Q * 256 :], AF.Tanh
                            )
                            for r in range(NROW):
                                grp = r & 3
                                bank = r >> 2
                                nc.tensor.matmul(
                                    sps[
                                        32 * grp : 32 * grp + 1,
                                        bank * 512 : (bank + 1) * 512,
                                    ],
                                    w2t[:, c : c + 1],
                                    tt[:, r * 512 : (r + 1) * 512],
                                    start=(c == 0),
                                    stop=(c == DC - 1),
                                    tile_position=(0, 32 * grp),
                                    skip_group_check=True,
                                )
                        # drain full psum block, bounce through DRAM to scatter
                        sdr = drp.tile([128, 4 * 512], F32, tag="sdr")
                        nc.vector.tensor_copy(sdr[:], sps[:])
                        for rl in range(4):
                            nc.sync.dma_start(
                                scr_d[qb][:, rl, :, :],
                                sdr[32 * rl : 32 * rl + 1, :].rearrange(
                                    "p (rh j k) -> p rh j k", rh=4, j=2
                                ),
                            )
                        nc.sync.dma_start(
                            S[qb * QPB : (qb + 1) * QPB, :],
                            scr_d[qb].rearrange("rh rl j k -> (rh rl j) k"),
                        )

            # ---- softmax over k (free dim); no max-subtraction needed ----
            with tc.tile_pool(name="smax", bufs=1) as smp:
                nc.vector.tensor_tensor(
                    S[:], S[:], mask_sb[:], mybir.AluOpType.add
                )
                nc.scalar.activation(S[:], S[:], AF.Exp)
                z = smp.tile([128, 1], F32)
                nc.vector.tensor_reduce(
                    z[:], S[:], mybir.AxisListType.X, mybir.AluOpType.add
                )
                r = smp.tile([128, 1], F32)
                nc.vector.reciprocal(r[:], z[:])
                nc.vector.tensor_scalar_mul(S[:], S[:], r[:, 0:1])

            # ---- prob transpose + ant matmul + outputs ----
            with (
                tc.tile_pool(
                    name="psum_t", bufs=2, space=bass.MemorySpace.PSUM
                ) as ptp,
                tc.tile_pool(name="tail", bufs=1) as tlp,
            ):
                pT = [tlp.tile([128, 128], F32, tag=f"pT{i}", name=f"pT{i}") for i in range(KB)]
                for kb in range(KB):
                    tp = ptp.tile([128, 128], F32, tag="tp2")
                    nc.tensor.transpose(
                        tp[:], S[:, kb * 128 : (kb + 1) * 128], ident[:]
                    )
                    nc.vector.tensor_copy(pT[kb][:], tp[:])
                    nc.sync.dma_start(prob_d[kb * 128 : (kb + 1) * 128, :], pT[kb][:])

                pant = ptp.tile([128, H], F32, tag="pant")
                for kb in range(KB):
                    nc.tensor.matmul(
                        pant[:],
                        pT[kb][:],
                        ck[kb][:],
                        start=(kb == 0),
                        stop=(kb == KB - 1),
                    )
                asb = tlp.tile([128, H], F32, tag="asb")
                nc.vector.tensor_copy(asb[:], pant[:])
                nc.sync.dma_start(ant_d[:], asb[:])

    nc.compile()
    return nc


_NC_CACHE = None


def _get_program():
    global _NC_CACHE
    if _NC_CACHE is None:
        _NC_CACHE = build_program()
    return _NC_CACHE


def make_in_maps(query, c_key, mask, W1, b1, W2, b2):
    query = np.asarray(query, dtype=np.float32)
    c_key = np.asarray(c_key, dtype=np.float32)
    mask = np.asarray(mask)
    W1 = np.asarray(W1, dtype=np.float32)
    b1 = np.asarray(b1, dtype=np.float32)
    W2 = np.asarray(W2, dtype=np.float32)

    w1a = np.ascontiguousarray(W1[:H])
    w1b = np.ascontiguousarray(W1[H:])
    w2t = np.ascontiguousarray(W2[:, 0].reshape(DC, 128).T)
    if TT_BF16:
        w2t = w2t.astype(ml_dtypes.bfloat16)
    b1t = np.ascontiguousarray(b1.reshape(DC, 128).T)
    ident = np.eye(128, dtype=np.float32)
    ones = np.ones((1, 128), dtype=np.float32)

    in_maps = []
    for b in range(NC):
        madd = np.where(mask[:, b], np.float32(NEG), np.float32(0.0)).astype(
            np.float32
        )
        in_maps.append(
            {
                "ck": np.ascontiguousarray(c_key[:, b, :]),
                "qin": np.ascontiguousarray(query[:, b, :]),
                "w1a": w1a,
                "w1b": w1b,
                "w2t": w2t,
                "b1t": b1t,
                "madd": madd[None, :],
                "ident": ident,
                "ones": ones,
            }
        )
    return in_maps


def assemble(results):
    ant = np.stack([results[b]["ant"] for b in range(NC)], axis=1)
    prob = np.stack([results[b]["prob_t"] for b in range(NC)], axis=-1)
    return ant, prob


def kernel(query, c_key, mask, W1, b1, W2, b2):
    nc = _get_program()
    in_maps = make_in_maps(query, c_key, mask, W1, b1, W2, b2)
    res = run_bass_kernel_spmd(nc, in_maps, list(range(NC))).results
    return assemble(res)
